# revision 33
# baseline (speedup 1.0000x reference)
"""Trainium2 Bass kernel for nn_MetaStateStep (decay-attention + GLU block).

Sharding: 8 cores = (batch b in 0..3) x (T-half h in 0..1). Each core
processes its 1024 own rows plus a 128-row lookahead halo (decay =
sigmoid(3); truncating lag>128 contributes ~1e-3 relative error, well
under the 2e-2 gate).

Key structure vs the straightforward version:
- Banded attention computed s-chunk-major: per s-chunk j the scores
  block [128 s x 384 t] covers exactly the nonzero decay band, halving
  both the PE score work and the DVE weighting muls.
- The GLU down/up projections run in fp8e4m3 DoubleRow. DoubleRow
  disables fast-weight-load so LDWEIGHTS (225ns) dominates the 512-col
  matmul (107ns); we pair the two T-halves per stationary and a
  post-schedule pass (dedup_ldweights) removes the second identical
  LDWEIGHTS so each fp8 stationary is loaded once per pair.
- rms-norm partition sums ride ones-stationary fp8 DoubleRow matmuls
  (one LDWEIGHTS for the whole chain after dedup).
- The final residual is folded into the up-projection PSUM via a
  256*I bf16 matmul (weights are pre-scaled x256 into fp8 range), so
  the output drain is a single scaled copy instead of a mul-add.
- DMA: x/w tiles stream on the Sync queue interleaved in consumption
  order (slab0 columns first) with the slab-0 accumulation loop
  vt-outer so the PE chases the DMA arrivals; all other inputs issue
  from the otherwise-idle GpSimd SWDGE; stores alternate Sync/Act.
"""

import numpy as np
import ml_dtypes

import concourse.bass as bass
import concourse.tile as tile
from concourse import bacc, mybir
from concourse.bass_utils import run_bass_kernel_spmd
from concourse import bass_utils

# avoid artifact uploads from the trace path if a caller enables tracing
bass_utils.upload_artifacts = lambda tmpdir: "local://" + tmpdir

F32 = mybir.dt.float32
BF16 = mybir.dt.bfloat16
F8 = mybir.dt.float8e4
DROW = mybir.MatmulPerfMode.DoubleRow
AF = mybir.ActivationFunctionType
NP_BF16 = ml_dtypes.bfloat16
NP_F8 = ml_dtypes.float8_e4m3fn
W8SCALE = 256.0

B, T, V = 4, 2048, 2048
D, R = 256, 512
C = 128
T_OWN = 1024
T_HALO = 128
T_TOT = T_OWN + T_HALO     # 1152
SLAB_W = [512, 512, T_HALO]
NVT = V // 128             # 16
NSC = T_TOT // 128         # 9 s-chunks
NTC = T_OWN // 128         # 8 own t-chunks
SCW = 384                  # score block t-width (3 chunks)
WCOL = 3 * D               # [wk | wq | wv]
EPS = float(np.finfo(np.float32).eps)

_NC_CACHE = {}


def _t0j(j):
    return min(max(j - 2, 0), NTC - 3)


def dedup_ldweights(nc):
    """Drop back-to-back identical LDWEIGHTS (same AP/perf-mode, no sync
    waits) so a stationary loaded once serves consecutive matmuls."""
    n_removed = 0
    for fn in nc.m.functions:
        for bb in fn.blocks:
            insts = list(bb.instructions)
            keep = []
            last_key = None
            for inst in insts:
                if isinstance(inst, mybir.InstLdweights):
                    key = (str(inst.ins[0]), str(inst.perf_mode),
                           str(inst.tile_position), str(inst.is_transpose),
                           str(inst.tile_size))
                    si = inst.sync_info
                    clean = si is None or (
                        len(si.on_wait) == 0 and len(si.on_update) == 0)
                    if key == last_key and clean:
                        n_removed += 1
                        continue
                    last_key = key
                elif isinstance(inst, mybir.InstMatmult):
                    if inst.is_transpose:
                        last_key = None
                keep.append(inst)
            if len(keep) != len(insts):
                bb.instructions.clear()
                for i in keep:
                    bb.instructions.append(i)
    return n_removed


def _build_nc():
    nc = bacc.Bacc("TRN2", target_bir_lowering=False, debug=False,
                   num_devices=8)

    xT = nc.dram_tensor("xT", [V, T_TOT], BF16, kind="ExternalInput")
    wkqvd = nc.dram_tensor("wkqvd", [V, WCOL], BF16, kind="ExternalInput")
    woT = nc.dram_tensor("woT", [D, V], BF16, kind="ExternalInput")
    wd8 = nc.dram_tensor("wd8", [NVT // 2, 128, 2 * R], F8, kind="ExternalInput")
    wu8 = nc.dram_tensor("wu8", [2, 128, 2 * V], F8, kind="ExternalInput")
    ww = nc.dram_tensor("ww", [NSC, C, SCW], BF16, kind="ExternalInput")
    ones8_d = nc.dram_tensor("ones8", [C, 2 * C], F8, kind="ExternalInput")
    eye_d = nc.dram_tensor("eye256", [C, C], BF16, kind="ExternalInput")
    tbias_d = nc.dram_tensor("tbias", [C, R // C], F32, kind="ExternalInput")
    eps_d = nc.dram_tensor("eps", [C, 2], F32, kind="ExternalInput")
    outT = nc.dram_tensor("outT", [V, T_OWN], BF16, kind="ExternalOutput")

    with tile.TileContext(nc) as tc:
        _emit(nc, tc, xT, wkqvd, woT, wd8, wu8, ww, ones8_d, eye_d,
              tbias_d, eps_d, outT)
    n = dedup_ldweights(nc)
    print(f"dedup_ldweights removed {n}")
    nc.finalize()
    return nc


def _emit(nc, tc, xT, wkqvd, woT, wd8, wu8, ww, ones8_d, eye_d,
          tbias_d, eps_d, outT):
    from contextlib import ExitStack

    ctx = ExitStack()
    with ctx:
        pers = ctx.enter_context(tc.tile_pool(name="pers", bufs=1))
        # shared by phase-A x squares and phase-C out1 squares
        sqpool = ctx.enter_context(tc.tile_pool(name="sq", bufs=16))
        vecpool = ctx.enter_context(tc.tile_pool(name="vecs", bufs=2))
        sb1pool = ctx.enter_context(tc.tile_pool(name="sb1p", bufs=3))
        finpool = ctx.enter_context(tc.tile_pool(name="fin", bufs=4))
        # PSUM: A x5 + B x2 + C x1 = 8 banks
        psA = ctx.enter_context(tc.tile_pool(name="psA", bufs=5, space="PSUM"))
        psB = ctx.enter_context(tc.tile_pool(name="psB", bufs=2, space="PSUM"))
        psC = ctx.enter_context(tc.tile_pool(name="psC", bufs=1, space="PSUM"))

        # ---- persistent SBUF tensors; gpsimd issues all non-x/w loads ----
        ones8_t = pers.tile([C, 2, C], F8, tag="ones8")
        nc.gpsimd.dma_start(ones8_t[:], ones8_d[:])
        eps_t = pers.tile([C, 2], F32, tag="eps")
        nc.gpsimd.dma_start(eps_t[:], eps_d[:])
        tbias_t = pers.tile([C, R // C], F32, tag="tbias")
        nc.gpsimd.dma_start(tbias_t[:], tbias_d[:])
        eye_t = pers.tile([C, C], BF16, tag="eye")
        nc.gpsimd.dma_start(eye_t[:], eye_d[:])

        # full-width x rows (2.3KB lines): DMA is descriptor-rate bound
        # (~87ns/line/engine), so fewer fat lines beat split loads; x on
        # Sync and w on Act drain in parallel, pairs arrive in vt order
        xs, wv_t = [], []
        for vt in range(NVT):
            x = pers.tile([128, T_TOT], BF16, tag=f"xs{vt}", name=f"xs{vt}")
            w = pers.tile([128, WCOL], BF16, tag=f"w{vt}", name=f"w{vt}")
            nc.sync.dma_start(x[:], xT[vt * 128:(vt + 1) * 128, :])
            nc.scalar.dma_start(w[:], wkqvd[vt * 128:(vt + 1) * 128, :])
            xs.append(x)
            wv_t.append(w)

        # later-phase weights queue on Sync BEHIND x so they don't steal
        # bus bandwidth from slab0/slab1 (queue drains in issue order)
        ww_t = []
        for j in range(NSC):
            w = pers.tile([C, SCW], BF16, tag=f"ww{j}", name=f"ww{j}")
            nc.sync.dma_start(w[:], ww[j])
            ww_t.append(w)
        wo_t, wd8_t, wu8_t = [], [], []
        for dh in range(2):
            w = pers.tile([128, V], BF16, tag=f"wo{dh}", name=f"wo{dh}")
            nc.sync.dma_start(w[:], woT[dh * 128:(dh + 1) * 128, :])
            wo_t.append(w)
        for k in range(NVT // 2):
            w = pers.tile([128, 2, R], F8, tag=f"wd8_{k}", name=f"wd8_{k}")
            nc.sync.dma_start(w[:], wd8[k])
            wd8_t.append(w)
        for rp in range(2):
            w = pers.tile([128, 2, V], F8, tag=f"wu8_{rp}", name=f"wu8_{rp}")
            nc.sync.dma_start(w[:], wu8[rp])
            wu8_t.append(w)

        kt = [pers.tile([128, T_TOT], BF16, tag=f"kt{dh}", name=f"kt{dh}")
              for dh in range(2)]
        qt = [pers.tile([128, T_OWN], BF16, tag=f"qt{dh}", name=f"qt{dh}")
              for dh in range(2)]
        vs = pers.tile([128, NSC, D], BF16, tag="vs", name="vs")
        out1 = [pers.tile([128, T_OWN], BF16, tag=f"o1_{vt}", name=f"o1_{vt}")
                for vt in range(NVT)]
        o18 = [pers.tile([128, 2, T_OWN], F8, tag=f"o18_{k}", name=f"o18_{k}")
               for k in range(NVT // 2)]
        hg8 = [pers.tile([128, 2, T_OWN], F8, tag=f"hg8_{rp}", name=f"hg8_{rp}")
               for rp in range(2)]
        retr = [[pers.tile([128, 512], BF16, tag=f"re{tb}{dh}",
                           name=f"re{tb}{dh}") for dh in range(2)]
                for tb in range(2)]
        n2b = pers.tile([128, T_OWN], BF16, tag="n2b")
        wsc = [pers.tile([128, SCW], BF16, tag=f"wsc{j}", name=f"wsc{j}")
               for j in range(NSC)]

        # =========== Phase A: q/k/v projections + norm ===========
        def emit_slab(slab):
            t0 = slab * 512
            w = SLAB_W[slab]
            nrc = w // 128
            pk = [psA.tile([128, 512], F32, tag="A", name=f"pk{dh}")
                  for dh in range(2)]
            pq = ([psA.tile([128, 512], F32, tag="A", name=f"pq{dh}")
                   for dh in range(2)] if slab < 2 else [])
            # two rc share one bank: exactly ONE start=True per bank clears
            # the whole 2KB zero region; the other region's first write is a
            # first-touch overwrite (has_written=0), later writes accumulate
            pv = [psB.tile([128, 512], F32, tag="B", name=f"pvb{b}")
                  for b in range((nrc + 1) // 2)]
            sqs = []
            for vt in range(NVT):
                if vt % 2 == 0:
                    cur = sqpool.tile([128, 2, 512], F8, tag="sq8")
                    sqs.append(cur)
                # squares: ACT for 512-slabs, DVE for the halo slab
                if slab < 2:
                    nc.scalar.activation(cur[:, vt % 2, :w],
                                         xs[vt][:, t0:t0 + w], AF.Square)
                else:
                    nc.vector.tensor_mul(cur[:, vt % 2, :w],
                                         xs[vt][:, t0:t0 + w],
                                         xs[vt][:, t0:t0 + w])
                for dh in range(2):
                    nc.tensor.matmul(pk[dh][:, :w],
                                     wv_t[vt][:, dh * 128:(dh + 1) * 128],
                                     xs[vt][:, t0:t0 + w], start=(vt == 0),
                                     stop=(vt == NVT - 1))
                if slab < 2:
                    for dh in range(2):
                        nc.tensor.matmul(
                            pq[dh][:],
                            wv_t[vt][:, D + dh * 128:D + (dh + 1) * 128],
                            xs[vt][:, t0:t0 + 512], start=(vt == 0),
                            stop=(vt == NVT - 1))
                for rc in range(nrc):
                    nc.tensor.matmul(
                        pv[rc // 2][:, (rc % 2) * D:(rc % 2 + 1) * D],
                        xs[vt][:, t0 + rc * 128:t0 + (rc + 1) * 128],
                        wv_t[vt][:, 2 * D:3 * D],
                        start=(vt == 0 and rc % 2 == 0),
                        stop=(vt == NVT - 1), skip_group_check=True)
            for b in range((nrc + 1) // 2):
                wd = min(512, (nrc - 2 * b) * D)
                nc.vector.tensor_copy(
                    vs[:, slab * 4 + 2 * b:slab * 4 + 2 * b + wd // D, :],
                    pv[b][:, :wd])
            # q's norm scale is folded into the retrieve drain (sb1 on the
            # surviving t axis), so pq drains as a plain copy right away
            if slab < 2:
                for dh in range(2):
                    nc.vector.tensor_copy(qt[dh][:, t0:t0 + 512], pq[dh][:])
            pn = psC.tile([128, 512], F32, tag="C", name="pn")
            for k in range(NVT // 2):
                nc.tensor.matmul(pn[:, :w], ones8_t[:], sqs[k][:, :, :w],
                                 start=(k == 0), stop=(k == NVT // 2 - 1),
                                 perf_mode=DROW)
            sb1 = sb1pool.tile([128, 512], F32, tag="sb1")
            nc.scalar.activation(sb1[:, :w], pn[:, :w], AF.Abs_reciprocal_sqrt,
                                 bias=eps_t[:, 0:1], scale=1.0 / V)
            sb2 = vecpool.tile([128, 512], BF16, tag="sb2")
            nc.scalar.activation(sb2[:, :w], sb1[:, :w], AF.Square)
            for dh in range(2):
                nc.vector.tensor_mul(kt[dh][:, t0:t0 + w], pk[dh][:, :w],
                                     sb2[:, :w])
            return sb1

        # =========== Phase B: banded decay attention ===========
        def emit_scores(js):
            for j in js:
                t0c = _t0j(j)
                psc = psB.tile([128, 512], F32, tag="B", name=f"psc{j}")
                for dh in range(2):
                    nc.tensor.matmul(psc[:, :SCW],
                                     kt[dh][:, j * 128:(j + 1) * 128],
                                     qt[dh][:, t0c * 128:t0c * 128 + SCW],
                                     start=(dh == 0), stop=(dh == 1))
                nc.vector.tensor_mul(wsc[j][:], psc[:, :SCW], ww_t[j][:])

        def emit_retrieve(tb, sb1_tb):
            # j-major (one LDWEIGHTS per vs slice via dedup); one start=True
            # per pr bank clears it, later regions first-touch overwrite
            pr = [psA.tile([128, 512], F32, tag="A", name=f"pr{tb}{dh}")
                  for dh in range(2)]
            for j in range(tb * 4, min(tb * 4 + 6, NSC)):
                for dh in range(2):
                    for tc in range(max(tb * 4, j - 2), min(j, tb * 4 + 3) + 1):
                        nc.tensor.matmul(
                            pr[dh][:, (tc % 4) * 128:(tc % 4 + 1) * 128],
                            vs[:, j, dh * 128:(dh + 1) * 128],
                            wsc[j][:, (tc - _t0j(j)) * 128:
                                   (tc - _t0j(j) + 1) * 128],
                            start=(j == tb * 4 and tc == tb * 4),
                            stop=(j == min(tc + 2, NSC - 1)),
                            skip_group_check=True)
            for dh in range(2):
                nc.vector.tensor_mul(retr[tb][dh][:], pr[dh][:], sb1_tb[:])

        # =========== Phase C: Wo + residual + GLU ===========
        def emit_fused(tb):
            # DVE does the residual adds (they pace the pat ring); squares
            # ride ACT straight from out1. fp8 casts: tb0's go to the slow
            # but idle gpsimd (19us of slack before down needs them), tb1's
            # run on DVE right after the adds so down isn't gpsimd-paced.
            t0 = tb * 512
            sq8s = []
            for vt in range(NVT):
                pat = psA.tile([128, 512], F32, tag="A", name="pat")
                for dh in range(2):
                    nc.tensor.matmul(pat[:],
                                     wo_t[dh][:, vt * 128:(vt + 1) * 128],
                                     retr[tb][dh][:], start=(dh == 0),
                                     stop=(dh == 1))
                nc.vector.tensor_add(out1[vt][:, t0:t0 + 512], pat[:],
                                     xs[vt][:, t0:t0 + 512])
                if vt % 2 == 0:
                    s8 = sqpool.tile([128, 2, 512], F8, tag="sq8")
                    sq8s.append(s8)
                nc.scalar.activation(s8[:, vt % 2, :],
                                     out1[vt][:, t0:t0 + 512], AF.Square)
            if tb == 1:
                # DVE fp8 casts after the adds, k-grouped so the down
                # projection's k-steps consume them in production order
                for k in range(NVT // 2):
                    for cvt in (2 * k, 2 * k + 1):
                        for ctb in range(2):
                            nc.vector.tensor_copy(
                                o18[cvt // 2][:, cvt % 2,
                                              ctb * 512:ctb * 512 + 512],
                                out1[cvt][:, ctb * 512:ctb * 512 + 512])
            return sq8s

        def emit_pn2(tb, sq8s):
            pn2 = psC.tile([128, 512], F32, tag="C", name=f"pn2_{tb}")
            for k in range(NVT // 2):
                nc.tensor.matmul(pn2[:], ones8_t[:], sq8s[k][:],
                                 start=(k == 0), stop=(k == NVT // 2 - 1),
                                 perf_mode=DROW)
            t0 = tb * 512
            nc.scalar.activation(n2b[:, t0:t0 + 512], pn2[:],
                                 AF.Abs_reciprocal_sqrt, bias=eps_t[:, 1:2],
                                 scale=W8SCALE * W8SCALE / V)

        def emit_down(rt):
            ph = [psA.tile([128, 512], F32, tag="A", name=f"ph{rt}{tb}")
                  for tb in range(2)]
            for k in range(NVT // 2):
                for tb in range(2):
                    nc.tensor.matmul(ph[tb][:],
                                     wd8_t[k][:, :, rt * 128:(rt + 1) * 128],
                                     o18[k][:, :, tb * 512:(tb + 1) * 512],
                                     start=(k == 0), stop=(k == NVT // 2 - 1),
                                     perf_mode=DROW)
            return ph

        def emit_neck(rt, ph):
            hpre = vecpool.tile([128, T_OWN], BF16, tag="hpre")
            for tb in range(2):
                nc.vector.tensor_mul(hpre[:, tb * 512:(tb + 1) * 512],
                                     ph[tb][:], n2b[:, tb * 512:(tb + 1) * 512])
            nc.scalar.activation(hg8[rt // 2][:, rt % 2, :], hpre[:],
                                 AF.Gelu, bias=tbias_t[:, rt:rt + 1])

        def emit_up():
            for vt in range(NVT):
                po = [psA.tile([128, 512], F32, tag="A", name=f"po{vt}{tb}")
                      for tb in range(2)]
                for rp in range(2):
                    for tb in range(2):
                        nc.tensor.matmul(
                            po[tb][:],
                            wu8_t[rp][:, :, vt * 128:(vt + 1) * 128],
                            hg8[rp][:, :, tb * 512:(tb + 1) * 512],
                            start=(rp == 0), stop=False, perf_mode=DROW)
                for tb in range(2):
                    # fold residual: po += 256 * out1  (eye_t is 256*I)
                    nc.tensor.matmul(po[tb][:], eye_t[:],
                                     out1[vt][:, tb * 512:(tb + 1) * 512],
                                     start=False, stop=True)
                fin = finpool.tile([128, T_OWN], BF16, tag="fin")
                nc.vector.tensor_scalar_mul(fin[:, 0:512], po[0][:],
                                            1.0 / W8SCALE)
                nc.scalar.activation(fin[:, 512:1024], po[1][:], AF.Copy,
                                     scale=1.0 / W8SCALE)
                nc.sync.dma_start(outT[vt * 128:(vt + 1) * 128, :], fin[:])

        # ---- schedule ----
        with nc.named_scope("slab0"):
            sb1_0 = emit_slab(0)
        with nc.named_scope("slab1"):
            sb1_1 = emit_slab(1)
        with nc.named_scope("scA"):
            emit_scores(range(0, 4))
        with nc.named_scope("slab2"):
            emit_slab(2)
        with nc.named_scope("scB"):
            emit_scores(range(4, NSC))
        with nc.named_scope("retr0"):
            emit_retrieve(0, sb1_0)
        with nc.named_scope("retr1"):
            emit_retrieve(1, sb1_1)
        with nc.named_scope("fused0"):
            sq80 = emit_fused(0)
        with nc.named_scope("fused1"):
            sq81 = emit_fused(1)
        with nc.named_scope("neck"):
            emit_pn2(0, sq80)
            emit_pn2(1, sq81)
            ph0 = emit_down(0)
            ph1 = emit_down(1)
            emit_neck(0, ph0)
            emit_neck(1, ph1)
            for rt in range(2, 4):
                ph = emit_down(rt)
                emit_neck(rt, ph)
        with nc.named_scope("up"):
            emit_up()


def _host_prep(inputs):
    x = np.asarray(inputs["x"], dtype=np.float32)
    Wq = np.asarray(inputs["Wq"], dtype=np.float32)
    Wk = np.asarray(inputs["Wk"], dtype=np.float32)
    Wv = np.asarray(inputs["Wv"], dtype=np.float32)
    Wo = np.asarray(inputs["Wo"], dtype=np.float32)
    Wdown = np.asarray(inputs["Wdown"], dtype=np.float32)
    Wup = np.asarray(inputs["Wup"], dtype=np.float32)
    t_bias = np.asarray(inputs["t_bias"], dtype=np.float32)
    decay_logit = float(np.asarray(inputs["decay_logit"]))
    q_out_scale = float(np.asarray(inputs["q_out_scale"]))
    t_out_scale = float(np.asarray(inputs["t_out_scale"]))
    q_scale = float(np.asarray(inputs["q_scale"]).reshape(-1)[0])
    t_scale = float(np.asarray(inputs["t_scale"]).reshape(-1)[0])

    decay = 1.0 / (1.0 + np.exp(-decay_logit))

    # banded decay weights, s-chunk major: ww[j][ss, lt] for
    # t_g = _t0j(j)*128 + lt, s_g = j*128 + ss, band o = j - tc in [0,2]
    ww = np.zeros((NSC, C, SCW), dtype=np.float32)
    ss = np.arange(C)[:, None].astype(np.float64)
    for j in range(NSC):
        t0c = _t0j(j)
        for lc in range(SCW // C):
            tc = t0c + lc
            o = j - tc
            if o < 0 or o > 2:
                continue
            tt = np.arange(C)[None, :].astype(np.float64)
            diff = o * C + ss - tt
            blk = np.where(diff > 0, decay ** np.clip(diff - 1.0, 0.0, None),
                           0.0)
            ww[j, :, lc * C:(lc + 1) * C] = blk.astype(np.float32)

    wkqvd = np.concatenate([Wk.T, Wq.T, Wv.T], axis=1)  # [V, 3D]
    WdT = np.ascontiguousarray(Wdown.T) * np.float32(W8SCALE)
    wd8 = np.zeros((NVT // 2, 128, 2 * R), dtype=np.float32)
    for k in range(NVT // 2):
        wd8[k, :, 0:R] = WdT[(2 * k) * 128:(2 * k + 1) * 128, :]
        wd8[k, :, R:2 * R] = WdT[(2 * k + 1) * 128:(2 * k + 2) * 128, :]
    WuT = (np.ascontiguousarray(Wup.T)
           * np.float32(t_scale * t_out_scale * W8SCALE))  # [R, V]
    wu8 = np.zeros((2, 128, 2 * V), dtype=np.float32)
    for rp in range(2):
        wu8[rp, :, 0:V] = WuT[(2 * rp) * 128:(2 * rp + 1) * 128, :]
        wu8[rp, :, V:2 * V] = WuT[(2 * rp + 1) * 128:(2 * rp + 2) * 128, :]

    shared = {
        "wkqvd": np.ascontiguousarray(wkqvd).astype(NP_BF16),
        "woT": (np.ascontiguousarray(Wo.T)
                * np.float32(q_scale * q_out_scale)).astype(NP_BF16),
        "wd8": wd8.astype(NP_F8),
        "wu8": wu8.astype(NP_F8),
        "ww": ww.astype(NP_BF16),
        "ones8": np.ones((C, 2 * C), np.float32).astype(NP_F8),
        "eye256": (np.eye(C, dtype=np.float32) * W8SCALE).astype(NP_BF16),
        "tbias": np.ascontiguousarray(t_bias.reshape(R // C, C).T),
        "eps": np.stack([np.full(C, EPS, np.float32),
                         np.full(C, EPS * W8SCALE * W8SCALE, np.float32)],
                        axis=1),
    }

    in_maps = []
    for core in range(8):
        b, h = core // 2, core % 2
        own = x[b, h * T_OWN:(h + 1) * T_OWN, :]
        if h == 0:
            halo = x[b, T_OWN:T_OWN + T_HALO, :]
        else:
            halo = np.zeros((T_HALO, V), np.float32)
        xT_c = np.ascontiguousarray(
            np.concatenate([own, halo], axis=0).T).astype(NP_BF16)
        m = dict(shared)
        m["xT"] = xT_c
        in_maps.append(m)
    return in_maps


def kernel(**inputs) -> np.ndarray:
    if "nc" not in _NC_CACHE:
        _NC_CACHE["nc"] = _build_nc()
    nc = _NC_CACHE["nc"]
    in_maps = _host_prep(inputs)
    res = run_bass_kernel_spmd(nc, in_maps, core_ids=list(range(8)))
    out = np.empty((B, T, V), np.float32)
    for core in range(8):
        b, h = core // 2, core % 2
        out[b, h * T_OWN:(h + 1) * T_OWN, :] = \
            res.results[core]["outT"].astype(np.float32).T
    return out


# revision 34
# speedup vs baseline: 1.1449x; 1.1449x over previous
"""Trainium2 Bass kernel for nn_MetaStateStep (decay-attention + GLU block).

Sharding: 8 cores = (batch b in 0..3) x (T-half h in 0..1). Each core
processes its 1024 own rows plus a 128-row lookahead halo (decay =
sigmoid(3); truncating lag>128 contributes ~1e-3 relative error, well
under the 2e-2 gate).

Key structure vs the straightforward version:
- Banded attention computed s-chunk-major: per s-chunk j the scores
  block [128 s x 384 t] covers exactly the nonzero decay band, halving
  both the PE score work and the DVE weighting muls.
- The GLU down/up projections run in fp8e4m3 DoubleRow. DoubleRow
  disables fast-weight-load so LDWEIGHTS (225ns) dominates the 512-col
  matmul (107ns); we pair the two T-halves per stationary and a
  post-schedule pass (dedup_ldweights) removes the second identical
  LDWEIGHTS so each fp8 stationary is loaded once per pair.
- rms-norm partition sums ride ones-stationary fp8 DoubleRow matmuls
  (one LDWEIGHTS for the whole chain after dedup).
- The final residual is folded into the up-projection PSUM via a
  256*I bf16 matmul (weights are pre-scaled x256 into fp8 range), so
  the output drain is a single scaled copy instead of a mul-add.
- DMA: x/w tiles stream on the Sync queue interleaved in consumption
  order (slab0 columns first) with the slab-0 accumulation loop
  vt-outer so the PE chases the DMA arrivals; all other inputs issue
  from the otherwise-idle GpSimd SWDGE; stores alternate Sync/Act.
"""

import numpy as np
import ml_dtypes

import concourse.bass as bass
import concourse.tile as tile
from concourse import bacc, mybir
from concourse.bass_utils import run_bass_kernel_spmd
from concourse import bass_utils

# avoid artifact uploads from the trace path if a caller enables tracing
bass_utils.upload_artifacts = lambda tmpdir: "local://" + tmpdir

F32 = mybir.dt.float32
BF16 = mybir.dt.bfloat16
F8 = mybir.dt.float8e4
DROW = mybir.MatmulPerfMode.DoubleRow
AF = mybir.ActivationFunctionType
NP_BF16 = ml_dtypes.bfloat16
NP_F8 = ml_dtypes.float8_e4m3fn
W8SCALE = 256.0

B, T, V = 4, 2048, 2048
D, R = 256, 512
C = 128
T_OWN = 1024
T_HALO = 128
T_TOT = T_OWN + T_HALO     # 1152
SLAB_W = [512, 512, T_HALO]
NVT = V // 128             # 16
NSC = T_TOT // 128         # 9 s-chunks
NTC = T_OWN // 128         # 8 own t-chunks
SCW = 384                  # score block t-width (3 chunks)
WCOL = 3 * D               # [wk | wq | wv]
EPS = float(np.finfo(np.float32).eps)

_NC_CACHE = {}


def _t0j(j):
    return min(max(j - 2, 0), NTC - 3)


def dedup_ldweights(nc):
    """Drop back-to-back identical LDWEIGHTS (same AP/perf-mode, no sync
    waits) so a stationary loaded once serves consecutive matmuls."""
    n_removed = 0
    for fn in nc.m.functions:
        for bb in fn.blocks:
            insts = list(bb.instructions)
            keep = []
            last_key = None
            for inst in insts:
                if isinstance(inst, mybir.InstLdweights):
                    key = (str(inst.ins[0]), str(inst.perf_mode),
                           str(inst.tile_position), str(inst.is_transpose),
                           str(inst.tile_size))
                    si = inst.sync_info
                    clean = si is None or (
                        len(si.on_wait) == 0 and len(si.on_update) == 0)
                    if key == last_key and clean:
                        n_removed += 1
                        continue
                    last_key = key
                elif isinstance(inst, mybir.InstMatmult):
                    if inst.is_transpose:
                        last_key = None
                keep.append(inst)
            if len(keep) != len(insts):
                bb.instructions.clear()
                for i in keep:
                    bb.instructions.append(i)
    return n_removed


def _build_nc():
    nc = bacc.Bacc("TRN2", target_bir_lowering=False, debug=False,
                   num_devices=8)

    xT = nc.dram_tensor("xT", [V, T_TOT], BF16, kind="ExternalInput")
    wkqvd = nc.dram_tensor("wkqvd", [V, WCOL], BF16, kind="ExternalInput")
    woT = nc.dram_tensor("woT", [D, V], BF16, kind="ExternalInput")
    wd8 = nc.dram_tensor("wd8", [NVT // 2, 128, 2 * R], F8, kind="ExternalInput")
    wu8 = nc.dram_tensor("wu8", [2, 128, 2 * V], F8, kind="ExternalInput")
    ww = nc.dram_tensor("ww", [NSC, C, SCW], BF16, kind="ExternalInput")
    ones8_d = nc.dram_tensor("ones8", [C, 2 * C], F8, kind="ExternalInput")
    eye_d = nc.dram_tensor("eye256", [C, C], BF16, kind="ExternalInput")
    tbias_d = nc.dram_tensor("tbias", [C, R // C], F32, kind="ExternalInput")
    eps_d = nc.dram_tensor("eps", [C, 2], F32, kind="ExternalInput")
    outT = nc.dram_tensor("outT", [V, T_OWN], BF16, kind="ExternalOutput")

    with tile.TileContext(nc) as tc:
        _emit(nc, tc, xT, wkqvd, woT, wd8, wu8, ww, ones8_d, eye_d,
              tbias_d, eps_d, outT)
    n = dedup_ldweights(nc)
    print(f"dedup_ldweights removed {n}")
    nc.finalize()
    return nc


def _emit(nc, tc, xT, wkqvd, woT, wd8, wu8, ww, ones8_d, eye_d,
          tbias_d, eps_d, outT):
    from contextlib import ExitStack

    ctx = ExitStack()
    with ctx:
        pers = ctx.enter_context(tc.tile_pool(name="pers", bufs=1))
        # shared by phase-A x squares and phase-C out1 squares
        sqpool = ctx.enter_context(tc.tile_pool(name="sq", bufs=16))
        vecpool = ctx.enter_context(tc.tile_pool(name="vecs", bufs=2))
        sb1pool = ctx.enter_context(tc.tile_pool(name="sb1p", bufs=3))
        finpool = ctx.enter_context(tc.tile_pool(name="fin", bufs=4))
        # PSUM: A x5 + B x2 + C x1 = 8 banks
        psA = ctx.enter_context(tc.tile_pool(name="psA", bufs=5, space="PSUM"))
        psB = ctx.enter_context(tc.tile_pool(name="psB", bufs=2, space="PSUM"))
        psC = ctx.enter_context(tc.tile_pool(name="psC", bufs=1, space="PSUM"))

        # ---- persistent SBUF tensors; gpsimd issues all non-x/w loads ----
        ones8_t = pers.tile([C, 2, C], F8, tag="ones8")
        nc.gpsimd.dma_start(ones8_t[:], ones8_d[:])
        eps_t = pers.tile([C, 2], F32, tag="eps")
        nc.gpsimd.dma_start(eps_t[:], eps_d[:])
        tbias_t = pers.tile([C, R // C], F32, tag="tbias")
        nc.gpsimd.dma_start(tbias_t[:], tbias_d[:])
        eye_t = pers.tile([C, C], BF16, tag="eye")
        nc.gpsimd.dma_start(eye_t[:], eye_d[:])

        xs, wv_t = [], []
        for vt in range(NVT):
            xs.append(pers.tile([128, T_TOT], BF16, tag=f"xs{vt}",
                                name=f"xs{vt}"))
            wv_t.append(pers.tile([128, WCOL], BF16, tag=f"w{vt}",
                                  name=f"w{vt}"))

        def xrest(vt):
            nc.sync.dma_start(xs[vt][:, 512:T_TOT],
                              xT[vt * 128:(vt + 1) * 128, 512:T_TOT])

        # slab0 x on Sync queue, w on Act queue: parallel issue, and the
        # bus serves slab0's needs first; xrest queues mostly behind
        # x-slab0 (a few slip in early so slab1's start isn't starved)
        for vt in range(NVT):
            nc.sync.dma_start(xs[vt][:, 0:512],
                              xT[vt * 128:(vt + 1) * 128, 0:512])
            nc.scalar.dma_start(wv_t[vt][:], wkqvd[vt * 128:(vt + 1) * 128, :])
            if 11 <= vt <= 14:
                xrest(vt - 11)
        for vt in range(4, NVT):
            xrest(vt)

        # later-phase weights queue on Sync BEHIND x so they don't steal
        # bus bandwidth from slab0/slab1 (queue drains in issue order)
        ww_t = []
        for j in range(NSC):
            w = pers.tile([C, SCW], BF16, tag=f"ww{j}", name=f"ww{j}")
            nc.sync.dma_start(w[:], ww[j])
            ww_t.append(w)
        wo_t, wd8_t, wu8_t = [], [], []
        for dh in range(2):
            w = pers.tile([128, V], BF16, tag=f"wo{dh}", name=f"wo{dh}")
            nc.sync.dma_start(w[:], woT[dh * 128:(dh + 1) * 128, :])
            wo_t.append(w)
        for k in range(NVT // 2):
            w = pers.tile([128, 2, R], F8, tag=f"wd8_{k}", name=f"wd8_{k}")
            nc.sync.dma_start(w[:], wd8[k])
            wd8_t.append(w)
        for rp in range(2):
            w = pers.tile([128, 2, V], F8, tag=f"wu8_{rp}", name=f"wu8_{rp}")
            nc.sync.dma_start(w[:], wu8[rp])
            wu8_t.append(w)

        kt = [pers.tile([128, T_TOT], BF16, tag=f"kt{dh}", name=f"kt{dh}")
              for dh in range(2)]
        qt = [pers.tile([128, T_OWN], BF16, tag=f"qt{dh}", name=f"qt{dh}")
              for dh in range(2)]
        vs = pers.tile([128, NSC, D], BF16, tag="vs", name="vs")
        out1 = [pers.tile([128, T_OWN], BF16, tag=f"o1_{vt}", name=f"o1_{vt}")
                for vt in range(NVT)]
        o18 = [pers.tile([128, 2, T_OWN], F8, tag=f"o18_{k}", name=f"o18_{k}")
               for k in range(NVT // 2)]
        hg8 = [pers.tile([128, 2, T_OWN], F8, tag=f"hg8_{rp}", name=f"hg8_{rp}")
               for rp in range(2)]
        retr = [[pers.tile([128, 512], BF16, tag=f"re{tb}{dh}",
                           name=f"re{tb}{dh}") for dh in range(2)]
                for tb in range(2)]
        n2b = pers.tile([128, T_OWN], BF16, tag="n2b")
        wsc = [pers.tile([128, SCW], BF16, tag=f"wsc{j}", name=f"wsc{j}")
               for j in range(NSC)]

        # =========== Phase A: q/k/v projections + norm ===========
        def emit_slab(slab):
            t0 = slab * 512
            w = SLAB_W[slab]
            nrc = w // 128
            pk = [psA.tile([128, 512], F32, tag="A", name=f"pk{dh}")
                  for dh in range(2)]
            pq = ([psA.tile([128, 512], F32, tag="A", name=f"pq{dh}")
                   for dh in range(2)] if slab < 2 else [])
            # two rc share one bank: exactly ONE start=True per bank clears
            # the whole 2KB zero region; the other region's first write is a
            # first-touch overwrite (has_written=0), later writes accumulate
            pv = [psB.tile([128, 512], F32, tag="B", name=f"pvb{b}")
                  for b in range((nrc + 1) // 2)]
            sqs = []
            for vt in range(NVT):
                if vt % 2 == 0:
                    cur = sqpool.tile([128, 2, 512], F8, tag="sq8")
                    sqs.append(cur)
                # squares: ACT for 512-slabs, DVE for the halo slab
                if slab < 2:
                    nc.scalar.activation(cur[:, vt % 2, :w],
                                         xs[vt][:, t0:t0 + w], AF.Square)
                else:
                    nc.vector.tensor_mul(cur[:, vt % 2, :w],
                                         xs[vt][:, t0:t0 + w],
                                         xs[vt][:, t0:t0 + w])
                for dh in range(2):
                    nc.tensor.matmul(pk[dh][:, :w],
                                     wv_t[vt][:, dh * 128:(dh + 1) * 128],
                                     xs[vt][:, t0:t0 + w], start=(vt == 0),
                                     stop=(vt == NVT - 1))
                if slab < 2:
                    for dh in range(2):
                        nc.tensor.matmul(
                            pq[dh][:],
                            wv_t[vt][:, D + dh * 128:D + (dh + 1) * 128],
                            xs[vt][:, t0:t0 + 512], start=(vt == 0),
                            stop=(vt == NVT - 1))
                for rc in range(nrc):
                    nc.tensor.matmul(
                        pv[rc // 2][:, (rc % 2) * D:(rc % 2 + 1) * D],
                        xs[vt][:, t0 + rc * 128:t0 + (rc + 1) * 128],
                        wv_t[vt][:, 2 * D:3 * D],
                        start=(vt == 0 and rc % 2 == 0),
                        stop=(vt == NVT - 1), skip_group_check=True)
            for b in range((nrc + 1) // 2):
                wd = min(512, (nrc - 2 * b) * D)
                nc.vector.tensor_copy(
                    vs[:, slab * 4 + 2 * b:slab * 4 + 2 * b + wd // D, :],
                    pv[b][:, :wd])
            # q's norm scale is folded into the retrieve drain (sb1 on the
            # surviving t axis), so pq drains as a plain copy right away
            if slab < 2:
                for dh in range(2):
                    nc.vector.tensor_copy(qt[dh][:, t0:t0 + 512], pq[dh][:])
            pn = psC.tile([128, 512], F32, tag="C", name="pn")
            for k in range(NVT // 2):
                nc.tensor.matmul(pn[:, :w], ones8_t[:], sqs[k][:, :, :w],
                                 start=(k == 0), stop=(k == NVT // 2 - 1),
                                 perf_mode=DROW)
            sb1 = sb1pool.tile([128, 512], F32, tag="sb1")
            nc.scalar.activation(sb1[:, :w], pn[:, :w], AF.Abs_reciprocal_sqrt,
                                 bias=eps_t[:, 0:1], scale=1.0 / V)
            sb2 = vecpool.tile([128, 512], BF16, tag="sb2")
            nc.scalar.activation(sb2[:, :w], sb1[:, :w], AF.Square)
            for dh in range(2):
                nc.vector.tensor_mul(kt[dh][:, t0:t0 + w], pk[dh][:, :w],
                                     sb2[:, :w])
            return sb1

        # =========== Phase B: banded decay attention ===========
        def emit_scores(js):
            for j in js:
                t0c = _t0j(j)
                psc = psB.tile([128, 512], F32, tag="B", name=f"psc{j}")
                for dh in range(2):
                    nc.tensor.matmul(psc[:, :SCW],
                                     kt[dh][:, j * 128:(j + 1) * 128],
                                     qt[dh][:, t0c * 128:t0c * 128 + SCW],
                                     start=(dh == 0), stop=(dh == 1))
                nc.vector.tensor_mul(wsc[j][:], psc[:, :SCW], ww_t[j][:])

        def emit_retrieve(tb, sb1_tb):
            # j-major (one LDWEIGHTS per vs slice via dedup); one start=True
            # per pr bank clears it, later regions first-touch overwrite
            pr = [psA.tile([128, 512], F32, tag="A", name=f"pr{tb}{dh}")
                  for dh in range(2)]
            for j in range(tb * 4, min(tb * 4 + 6, NSC)):
                for dh in range(2):
                    for tc in range(max(tb * 4, j - 2), min(j, tb * 4 + 3) + 1):
                        nc.tensor.matmul(
                            pr[dh][:, (tc % 4) * 128:(tc % 4 + 1) * 128],
                            vs[:, j, dh * 128:(dh + 1) * 128],
                            wsc[j][:, (tc - _t0j(j)) * 128:
                                   (tc - _t0j(j) + 1) * 128],
                            start=(j == tb * 4 and tc == tb * 4),
                            stop=(j == min(tc + 2, NSC - 1)),
                            skip_group_check=True)
            for dh in range(2):
                nc.vector.tensor_mul(retr[tb][dh][:], pr[dh][:], sb1_tb[:])

        # =========== Phase C: Wo + residual + GLU ===========
        def emit_fused(tb):
            # DVE does the residual adds (they pace the pat ring); squares
            # ride ACT straight from out1. fp8 casts: tb0's go to the slow
            # but idle gpsimd (19us of slack before down needs them), tb1's
            # run on DVE right after the adds so down isn't gpsimd-paced.
            t0 = tb * 512
            sq8s = []
            for vt in range(NVT):
                pat = psA.tile([128, 512], F32, tag="A", name="pat")
                for dh in range(2):
                    nc.tensor.matmul(pat[:],
                                     wo_t[dh][:, vt * 128:(vt + 1) * 128],
                                     retr[tb][dh][:], start=(dh == 0),
                                     stop=(dh == 1))
                nc.vector.tensor_add(out1[vt][:, t0:t0 + 512], pat[:],
                                     xs[vt][:, t0:t0 + 512])
                if vt % 2 == 0:
                    s8 = sqpool.tile([128, 2, 512], F8, tag="sq8")
                    sq8s.append(s8)
                nc.scalar.activation(s8[:, vt % 2, :],
                                     out1[vt][:, t0:t0 + 512], AF.Square)
            if tb == 1:
                # DVE fp8 casts after the adds, k-grouped so the down
                # projection's k-steps consume them in production order
                for k in range(NVT // 2):
                    for cvt in (2 * k, 2 * k + 1):
                        for ctb in range(2):
                            nc.vector.tensor_copy(
                                o18[cvt // 2][:, cvt % 2,
                                              ctb * 512:ctb * 512 + 512],
                                out1[cvt][:, ctb * 512:ctb * 512 + 512])
            return sq8s

        def emit_pn2(tb, sq8s):
            pn2 = psC.tile([128, 512], F32, tag="C", name=f"pn2_{tb}")
            for k in range(NVT // 2):
                nc.tensor.matmul(pn2[:], ones8_t[:], sq8s[k][:],
                                 start=(k == 0), stop=(k == NVT // 2 - 1),
                                 perf_mode=DROW)
            t0 = tb * 512
            nc.scalar.activation(n2b[:, t0:t0 + 512], pn2[:],
                                 AF.Abs_reciprocal_sqrt, bias=eps_t[:, 1:2],
                                 scale=W8SCALE * W8SCALE / V)

        def emit_down(rt):
            ph = [psA.tile([128, 512], F32, tag="A", name=f"ph{rt}{tb}")
                  for tb in range(2)]
            for k in range(NVT // 2):
                for tb in range(2):
                    nc.tensor.matmul(ph[tb][:],
                                     wd8_t[k][:, :, rt * 128:(rt + 1) * 128],
                                     o18[k][:, :, tb * 512:(tb + 1) * 512],
                                     start=(k == 0), stop=(k == NVT // 2 - 1),
                                     perf_mode=DROW)
            return ph

        def emit_neck(rt, ph):
            hpre = vecpool.tile([128, T_OWN], BF16, tag="hpre")
            for tb in range(2):
                nc.vector.tensor_mul(hpre[:, tb * 512:(tb + 1) * 512],
                                     ph[tb][:], n2b[:, tb * 512:(tb + 1) * 512])
            nc.scalar.activation(hg8[rt // 2][:, rt % 2, :], hpre[:],
                                 AF.Gelu, bias=tbias_t[:, rt:rt + 1])

        def emit_up():
            for vt in range(NVT):
                po = [psA.tile([128, 512], F32, tag="A", name=f"po{vt}{tb}")
                      for tb in range(2)]
                for rp in range(2):
                    for tb in range(2):
                        nc.tensor.matmul(
                            po[tb][:],
                            wu8_t[rp][:, :, vt * 128:(vt + 1) * 128],
                            hg8[rp][:, :, tb * 512:(tb + 1) * 512],
                            start=(rp == 0), stop=False, perf_mode=DROW)
                for tb in range(2):
                    # fold residual: po += 256 * out1  (eye_t is 256*I)
                    nc.tensor.matmul(po[tb][:], eye_t[:],
                                     out1[vt][:, tb * 512:(tb + 1) * 512],
                                     start=False, stop=True)
                fin = finpool.tile([128, T_OWN], BF16, tag="fin")
                nc.vector.tensor_scalar_mul(fin[:, 0:512], po[0][:],
                                            1.0 / W8SCALE)
                nc.scalar.activation(fin[:, 512:1024], po[1][:], AF.Copy,
                                     scale=1.0 / W8SCALE)
                nc.sync.dma_start(outT[vt * 128:(vt + 1) * 128, :], fin[:])

        # ---- schedule ----
        with nc.named_scope("slab0"):
            sb1_0 = emit_slab(0)
        with nc.named_scope("slab1"):
            sb1_1 = emit_slab(1)
        with nc.named_scope("scA"):
            emit_scores(range(0, 4))
        with nc.named_scope("slab2"):
            emit_slab(2)
        with nc.named_scope("scB"):
            emit_scores(range(4, NSC))
        with nc.named_scope("retr0"):
            emit_retrieve(0, sb1_0)
        with nc.named_scope("retr1"):
            emit_retrieve(1, sb1_1)
        with nc.named_scope("fused0"):
            sq80 = emit_fused(0)
        with nc.named_scope("fused1"):
            sq81 = emit_fused(1)
        with nc.named_scope("neck"):
            emit_pn2(0, sq80)
            emit_pn2(1, sq81)
            ph0 = emit_down(0)
            ph1 = emit_down(1)
            emit_neck(0, ph0)
            emit_neck(1, ph1)
            for rt in range(2, 4):
                ph = emit_down(rt)
                emit_neck(rt, ph)
        with nc.named_scope("up"):
            emit_up()


def _host_prep(inputs):
    x = np.asarray(inputs["x"], dtype=np.float32)
    Wq = np.asarray(inputs["Wq"], dtype=np.float32)
    Wk = np.asarray(inputs["Wk"], dtype=np.float32)
    Wv = np.asarray(inputs["Wv"], dtype=np.float32)
    Wo = np.asarray(inputs["Wo"], dtype=np.float32)
    Wdown = np.asarray(inputs["Wdown"], dtype=np.float32)
    Wup = np.asarray(inputs["Wup"], dtype=np.float32)
    t_bias = np.asarray(inputs["t_bias"], dtype=np.float32)
    decay_logit = float(np.asarray(inputs["decay_logit"]))
    q_out_scale = float(np.asarray(inputs["q_out_scale"]))
    t_out_scale = float(np.asarray(inputs["t_out_scale"]))
    q_scale = float(np.asarray(inputs["q_scale"]).reshape(-1)[0])
    t_scale = float(np.asarray(inputs["t_scale"]).reshape(-1)[0])

    decay = 1.0 / (1.0 + np.exp(-decay_logit))

    # banded decay weights, s-chunk major: ww[j][ss, lt] for
    # t_g = _t0j(j)*128 + lt, s_g = j*128 + ss, band o = j - tc in [0,2]
    ww = np.zeros((NSC, C, SCW), dtype=np.float32)
    ss = np.arange(C)[:, None].astype(np.float64)
    for j in range(NSC):
        t0c = _t0j(j)
        for lc in range(SCW // C):
            tc = t0c + lc
            o = j - tc
            if o < 0 or o > 2:
                continue
            tt = np.arange(C)[None, :].astype(np.float64)
            diff = o * C + ss - tt
            blk = np.where(diff > 0, decay ** np.clip(diff - 1.0, 0.0, None),
                           0.0)
            ww[j, :, lc * C:(lc + 1) * C] = blk.astype(np.float32)

    wkqvd = np.concatenate([Wk.T, Wq.T, Wv.T], axis=1)  # [V, 3D]
    WdT = np.ascontiguousarray(Wdown.T) * np.float32(W8SCALE)
    wd8 = np.zeros((NVT // 2, 128, 2 * R), dtype=np.float32)
    for k in range(NVT // 2):
        wd8[k, :, 0:R] = WdT[(2 * k) * 128:(2 * k + 1) * 128, :]
        wd8[k, :, R:2 * R] = WdT[(2 * k + 1) * 128:(2 * k + 2) * 128, :]
    WuT = (np.ascontiguousarray(Wup.T)
           * np.float32(t_scale * t_out_scale * W8SCALE))  # [R, V]
    wu8 = np.zeros((2, 128, 2 * V), dtype=np.float32)
    for rp in range(2):
        wu8[rp, :, 0:V] = WuT[(2 * rp) * 128:(2 * rp + 1) * 128, :]
        wu8[rp, :, V:2 * V] = WuT[(2 * rp + 1) * 128:(2 * rp + 2) * 128, :]

    shared = {
        "wkqvd": np.ascontiguousarray(wkqvd).astype(NP_BF16),
        "woT": (np.ascontiguousarray(Wo.T)
                * np.float32(q_scale * q_out_scale)).astype(NP_BF16),
        "wd8": wd8.astype(NP_F8),
        "wu8": wu8.astype(NP_F8),
        "ww": ww.astype(NP_BF16),
        "ones8": np.ones((C, 2 * C), np.float32).astype(NP_F8),
        "eye256": (np.eye(C, dtype=np.float32) * W8SCALE).astype(NP_BF16),
        "tbias": np.ascontiguousarray(t_bias.reshape(R // C, C).T),
        "eps": np.stack([np.full(C, EPS, np.float32),
                         np.full(C, EPS * W8SCALE * W8SCALE, np.float32)],
                        axis=1),
    }

    in_maps = []
    for core in range(8):
        b, h = core // 2, core % 2
        own = x[b, h * T_OWN:(h + 1) * T_OWN, :]
        if h == 0:
            halo = x[b, T_OWN:T_OWN + T_HALO, :]
        else:
            halo = np.zeros((T_HALO, V), np.float32)
        xT_c = np.ascontiguousarray(
            np.concatenate([own, halo], axis=0).T).astype(NP_BF16)
        m = dict(shared)
        m["xT"] = xT_c
        in_maps.append(m)
    return in_maps


def kernel(**inputs) -> np.ndarray:
    if "nc" not in _NC_CACHE:
        _NC_CACHE["nc"] = _build_nc()
    nc = _NC_CACHE["nc"]
    in_maps = _host_prep(inputs)
    res = run_bass_kernel_spmd(nc, in_maps, core_ids=list(range(8)))
    out = np.empty((B, T, V), np.float32)
    for core in range(8):
        b, h = core // 2, core % 2
        out[b, h * T_OWN:(h + 1) * T_OWN, :] = \
            res.results[core]["outT"].astype(np.float32).T
    return out
